# revision 22
# baseline (speedup 1.0000x reference)
"""GRU cell on 8 Trainium2 NeuronCores.

Reference computation (B=65536, D=256):
    z = sigmoid(x@Wz + h@Uz + bz)
    r = sigmoid(x@Wr + h@Ur + br)
    h_hat = tanh(x@Wh + (r*h)@Uh + bh)
    h_t = z*h + (1-z)*h_hat  ; returns (h_t, h_t)

Strategy: data-parallel over the batch dim (8 shards of 8192 rows), all
fp16 on chip (rel_l2 ~1.1e-3 vs the f32 reference; gate is 2e-2).  The
matmul stream runs at the fp16 PE issue floor (24 matmul passes over
8192 batch cols = 196608 PE cycles = 81.9us at 2.4GHz), so the
remaining time is the fixed framework preamble (~7.2us), the initial
HBM fill, and the drain tail + postamble.  Key structure:
  * host packs each shard as [128 partitions, 4 blocks, 8192] fp16
    (blocks = x k0, x k1, h k0, h k1): the contraction dim of all six
    GEMMs is the SBUF partition dim, fp16 halves HBM traffic and
    streams the PE at full rate with fast weight load.
  * all input tiles are SBUF-resident (8.4MB of 24MB); tiles are 512+
    cols so per-partition DMA lines stay at/above the 1KB efficiency
    knee (256-col tiles fragment to 512B lines and halve effective DMA
    bandwidth - measured).  Head DMAs are spread over the sync/scalar
    HWDGE rings (FIFO per ring, ~8 completion-sem lanes total) and
    gpsimd's SWDGE ring in need-order; the bulk stream self-paces on
    sync's lane rotation.
  * both ACT tables (sigmoid+tanh) are force-loaded via dummy
    activations right after scalar's head triggers - lazily, the tanh
    table load (1.28us) sits mid-queue and stalls the first r-sigmoid
    (measured as candidate-matmul stalls).
  * dummy warm-up matmuls during the head DMAs hold the PE's HAM clock
    gate at 2.4GHz so the real stream starts warm; ~3.4us of PE-busy
    is needed for the 1.2->2.4GHz flip, and any PE idle gap >~1us
    before the flip risks a cold stretch (measured: a 2us gap cost
    ~3us of half-rate matmuls).
  * the r-gate of work item i+1 is computed one iteration early so its
    sigmoid + r*h (ScalarE+VectorE) never gate the candidate matmuls.
  * the last 512 cols split into two 256-col pieces on disjoint PSUM
    regions; both use the short combine (u=z-1, m1=z*h precomputed on
    DVE, so only v=u*hh, o=m1-v remain after the tanh) and the last
    piece is emitted z-gate-first + candidate g-split so after the
    very last matmul only tanh(g1)+2 DVE ops+store remain.  Tail
    stores avoid scalar's queue except the final g1 (a 0.65us store
    trigger between the final tanhs measurably delays the drain).
"""

import os
import sys

for _p in ("/opt/trn_rl_repo", "/root/.axon_site/_ro/trn_rl_repo"):
    if os.path.isdir(_p) and _p not in sys.path:
        sys.path.append(_p)

import numpy as np

B = 65536
D = 256
N_CORES = 8
S = B // N_CORES  # batch rows per core
CH = 512  # batch columns per PSUM bank / compute sub-chunk

# Input-tile load plan: (col_start, width).  The first two are per-block
# (pipeline head fill); the rest are packed 4-block loads.
PLAN = [(0, 512), (512, 512), (1024, 512), (1536, 512)] + [
    (2048 + 1024 * i, 1024) for i in range(6)
]
_BLOCKS = ("x0", "x1", "h0", "h1")
_WORDER = ("Wr", "Ur", "Wz", "Uz", "Wh", "Uh")

# Work items: (dram col start, width, psum col offset).
WI = [(i * CH, CH, 0) for i in range(15)] + [
    (7680, 256, 0),
    (7936, 256, 256),
]


def build_nc(s=S, mm_dtype_name=None):
    """Build + compile the per-core Bass program for a shard of s rows."""
    import concourse.bass as bass
    import concourse.mybir as mybir
    import concourse.tile as tile
    from concourse import bacc

    f32 = mybir.dt.float32
    if mm_dtype_name is None:
        mm_dtype_name = os.environ.get("GRU_MM_DTYPE", "float16")
    f16 = getattr(mybir.dt, mm_dtype_name)
    AF = mybir.ActivationFunctionType
    n_warm = int(os.environ.get("GRU_WARMUP", "10"))

    nc = bacc.Bacc("TRN2", target_bir_lowering=False)
    xh = nc.dram_tensor("xh", [128, 4, s], f16, kind="ExternalInput")
    wcat = nc.dram_tensor("wcat", [D, 6 * D], f16, kind="ExternalInput")
    bcat = nc.dram_tensor("bcat", [128, 6], f32, kind="ExternalInput")
    outT = nc.dram_tensor("outT", [D, s], f16, kind="ExternalOutput")

    nwi = len(WI)

    with tile.TileContext(nc) as tc:
        with (
            tc.tile_pool(name="const", bufs=1) as cpool,
            tc.tile_pool(name="work", bufs=2) as wpool,
            tc.tile_pool(name="outb", bufs=4) as opool,
            tc.tile_pool(name="psum", bufs=1, space=bass.MemorySpace.PSUM) as ppool,
        ):
            inp = {}  # (block, load_idx) -> AP [128, width]

            # PE warm-up: the HAM clock gate needs ~3.4us of sustained PE
            # activity to lift the engine from 1.2 to 2.4 GHz.  The PE is
            # idle during the head DMAs anyway, so burn that window on
            # dummy matmuls over a memset tile.
            zt0 = cpool.tile([128, CH], f16, tag="warm", name="warm")
            nc.gpsimd.memset(zt0[:], 0)
            pw = ppool.tile([128, CH], f32, tag="pwarm", name="pwarm")
            for _ in range(n_warm):
                nc.tensor.matmul(pw[:], zt0[:, 0:128], zt0[:], start=True, stop=True)
            wsink = cpool.tile([128, CH], f32, tag="wsink", name="wsink")
            nc.vector.tensor_copy(wsink[:], pw[:])

            # Head DMA scheduling: HWDGE rings are FIFO per issuing engine
            # with ~8 completion-sem lanes; a trigger past that blocks its
            # engine queue until an earlier DMA completes.  The first slots
            # carry exactly the critical set (r-gate weights + j=0 input
            # tiles); h tiles and stragglers ride GpSimd's SWDGE ring; the
            # bulk stream self-paces on sync's lane rotation.
            wA, wZ, wH = {}, {}, {}
            for k in range(2):
                wA[k] = cpool.tile([128, 2 * D], f16, tag=f"wA{k}", name=f"wA{k}")
                wZ[k] = cpool.tile([128, 2 * D], f16, tag=f"wZ{k}", name=f"wZ{k}")
                wH[k] = cpool.tile([128, 2 * D], f16, tag=f"wH{k}", name=f"wH{k}")

            def load_block(blk, li, eng):
                bi = _BLOCKS.index(blk)
                start, width = PLAN[li]
                t = cpool.tile([128, width], f16, tag=f"i{blk}_{li}",
                               name=f"i{blk}_{li}")
                eng.dma_start(t[:], xh[:, bi, start : start + width])
                inp[(blk, li)] = t[:]

            # wave 1: r-gate weights and the j=0 input tiles - exactly what
            # the first matmuls consume - plus the tiny bias vector
            nc.sync.dma_start(wA[0][:], wcat[0:128, 0 : 2 * D])
            nc.scalar.dma_start(wA[1][:], wcat[128:256, 0 : 2 * D])
            load_block("x0", 0, nc.sync)
            load_block("x1", 0, nc.scalar)
            load_block("h0", 0, nc.gpsimd)
            load_block("h1", 0, nc.gpsimd)
            b_sb = cpool.tile([128, 6], f32, tag="bcat")
            nc.sync.dma_start(b_sb[:], bcat[:, :])
            # z/candidate weights split per-gate so the z chunk lands
            # before the z matmuls need it (a single 4D-wide wB chunk
            # finishes ~2us after the z matmuls want its first half).
            # Both wH chunks ride scalar: its HWDGE ring is consistently
            # ~1.5x faster than sync's, and wH as sync's 5th entry missed
            # its deadline by 1.5-3us across runs.
            nc.scalar.dma_start(wZ[0][:], wcat[0:128, 2 * D : 4 * D])
            nc.sync.dma_start(wZ[1][:], wcat[128:256, 2 * D : 4 * D])
            nc.scalar.dma_start(wH[0][:], wcat[0:128, 4 * D : 6 * D])
            nc.scalar.dma_start(wH[1][:], wcat[128:256, 4 * D : 6 * D])
            # force both ACT tables (sigmoid + tanh) to load now, after
            # scalar's immediate triggers but BEFORE the rotation-blocked
            # stragglers (which hold scalar's queue until earlier DMAs
            # complete): lazily the tanh table load lands after them and
            # stalls the first r-sigmoid by ~1.5us
            dume = cpool.tile([128, 1], f16, tag="dume", name="dume")
            nc.scalar.activation(dume[:], zt0[:, 0:1], AF.Sigmoid)
            nc.scalar.activation(dume[:], zt0[:, 0:1], AF.Tanh)
            # rotation-paced stragglers: these throttle themselves on their
            # engine's completion-sem lanes, so they cannot steal HBM
            # bandwidth from the critical wA/wB/x/h pieces above (measured:
            # moving them to gpsimd's free lanes delayed wB[0] by 2.3us)
            load_block("x0", 1, nc.sync)
            load_block("h0", 1, nc.sync)
            load_block("x1", 1, nc.scalar)
            load_block("h1", 1, nc.scalar)
            for li in range(2, len(PLAN)):
                start, width = PLAN[li]
                t = cpool.tile([128, 4, width], f16, tag=f"ixh_{li}",
                               name=f"ixh_{li}")
                nc.sync.dma_start(t[:], xh[:, :, start : start + width])
                for bi, blk in enumerate(_BLOCKS):
                    inp[(blk, li)] = t[:, bi, :]

            def wap(i, k, g):
                """Weight AP [128,128] for matrix index i (order _WORDER),
                contraction half k, output-feature half g."""
                chunk = (wA, wA, wZ, wZ, wH, wH)[i]
                return chunk[k][:, (i % 2) * D + g * 128 : (i % 2) * D + (g + 1) * 128]

            def inp_ap(blk, c0, w):
                for li, (start, width) in enumerate(PLAN):
                    if start <= c0 and c0 + w <= start + width:
                        return inp[(blk, li)][:, c0 - start : c0 - start + w]
                raise ValueError((blk, c0, w))

            def operands(i):
                c0, w, _ = WI[i]
                xs = [inp_ap(f"x{k}", c0, w) for k in range(2)]
                hs = [inp_ap(f"h{k}", c0, w) for k in range(2)]
                return xs, hs

            def gate_pair(tagbase, wi, ui, xs, rhs_u, po, w):
                """Both g-halves of one gate.  W (x-side) matmuls of both
                halves run before the U matmuls; k-major within each pass."""
                ps = []
                for g in range(2):
                    p = ppool.tile([128, CH], f32, tag=f"{tagbase}{g}",
                                   name=f"{tagbase}{g}")
                    ps.append(p)
                for k in range(2):
                    for g in range(2):
                        nc.tensor.matmul(ps[g][:, po : po + w], wap(wi, k, g),
                                         xs[k], start=(k == 0), stop=False)
                for k in range(2):
                    for g in range(2):
                        nc.tensor.matmul(ps[g][:, po : po + w], wap(ui, k, g),
                                         rhs_u[k], start=False, stop=(k == 1))
                return ps

            def r_gate(i):
                """reset gate -> r*h tiles for work item i."""
                c0, w, po = WI[i]
                xs, hs = operands(i)
                ps = gate_pair("pr", 0, 1, xs, hs, po, w)
                rh = []
                for g in range(2):
                    rt = wpool.tile([128, CH], f16, tag=f"r{g}", name=f"r{g}")
                    nc.scalar.activation(rt[:, 0:w], ps[g][:, po : po + w],
                                         AF.Sigmoid, bias=b_sb[:, g : g + 1])
                    t = wpool.tile([128, CH], f16, tag=f"rh{g}", name=f"rh{g}")
                    nc.vector.tensor_mul(t[:, 0:w], rt[:, 0:w], hs[g])
                    rh.append(t[:, 0:w])
                return rh

            def zu_gate(i, emit_um):
                """z-sigmoids for item i; with emit_um also u=z-1, m1=z*h
                on DVE (NOT GpSimd: its tensor ops cost ~3.8us each plus a
                9us drain penalty - measured) so only two DVE links remain
                after the final tanh: o = z*h+(1-z)*hh = m1-u*hh."""
                c0, w, po = WI[i]
                xs, hs = operands(i)
                pz = gate_pair("pz", 2, 3, xs, hs, po, w)
                zt, ut, m1 = [], [], []
                for g in range(2):
                    t = wpool.tile([128, CH], f16, tag=f"z{g}", name=f"z{g}")
                    nc.scalar.activation(t[:, 0:w], pz[g][:, po : po + w],
                                         AF.Sigmoid, bias=b_sb[:, 2 + g : 3 + g])
                    zt.append(t)
                    if emit_um:
                        u = wpool.tile([128, CH], f16, tag=f"u{g}", name=f"u{g}")
                        nc.vector.tensor_scalar_sub(u[:, 0:w], t[:, 0:w], 1.0)
                        ut.append(u)
                        m = wpool.tile([128, CH], f16, tag=f"zh{g}", name=f"zh{g}")
                        nc.vector.tensor_mul(m[:, 0:w], t[:, 0:w], hs[g])
                        m1.append(m)
                return zt, ut, m1

            def combine_short(g, ut, m1, hh, w):
                v = wpool.tile([128, CH], f16, tag=f"v{g}", name=f"v{g}")
                nc.vector.tensor_mul(v[:, 0:w], ut[g][:, 0:w], hh[:, 0:w])
                o = opool.tile([128, CH], f16, tag=f"o{g}", name=f"o{g}")
                nc.vector.tensor_sub(o[:, 0:w], m1[g][:, 0:w], v[:, 0:w])
                return o

            # software pipeline: r-gate one work item ahead of z/candidate;
            # per-iteration order is z_i, r_{i+1}, h_i so the r_i sigmoid +
            # r*h chain gets a 16-matmul window before h_i consumes it
            # (uniform from i=0 on - prefetching r_1 inside item 0 shrinks
            # the pipeline-fill bubble by ~1us).
            rh_cur = r_gate(0)
            for i in range(nwi - 1):
                c0, w, po = WI[i]
                xs, hs = operands(i)
                tail = i == nwi - 2
                zt, ut, m1 = zu_gate(i, False)
                rh_next = r_gate(i + 1) if i + 1 < nwi else None
                ph = gate_pair("ph", 4, 5, xs, rh_cur, po, w)
                for g in range(2):
                    hh = wpool.tile([128, CH], f16, tag=f"hh{g}", name=f"hh{g}")
                    nc.scalar.activation(hh[:, 0:w], ph[g][:, po : po + w],
                                         AF.Tanh, bias=b_sb[:, 4 + g : 5 + g])
                    o = opool.tile([128, CH], f16, tag=f"o{g}", name=f"o{g}")
                    d = wpool.tile([128, CH], f16, tag=f"d{g}", name=f"d{g}")
                    nc.vector.tensor_sub(d[:, 0:w], hs[g], hh[:, 0:w])
                    m = wpool.tile([128, CH], f16, tag=f"m{g}", name=f"m{g}")
                    nc.vector.tensor_mul(m[:, 0:w], zt[g][:, 0:w], d[:, 0:w])
                    nc.vector.tensor_add(o[:, 0:w], hh[:, 0:w], m[:, 0:w])
                    orow = outT[g * 128 : (g + 1) * 128, :]
                    # bulk stores ride gpsimd's SWDGE ring.  The second-to-
                    # last item's g0 goes on sync, g1 on gpsimd: both sync-
                    # both would rotation-block sync's queue ahead of the
                    # LAST item's store trigger (measured +1.5us), and
                    # scalar must stay clear for the final tanhs.
                    if tail:
                        eng = nc.sync if g == 0 else nc.gpsimd
                    else:
                        eng = nc.gpsimd
                    eng.dma_start(orow[:, c0 : c0 + w], o[:, 0:w])
                rh_cur = rh_next

            # Last item, drain-optimized: z first (sigmoid + u/m1 while the
            # candidate matmuls run), candidate g-split so after the very
            # last matmul only tanh(g1) -> v -> o -> store remain.
            i = nwi - 1
            c0, w, po = WI[i]
            xs, hs = operands(i)
            zt, ut, m1 = zu_gate(i, True)
            for g in range(2):
                p = ppool.tile([128, CH], f32, tag=f"ph{g}", name=f"ph{g}")
                for k in range(2):
                    nc.tensor.matmul(p[:, po : po + w], wap(4, k, g), xs[k],
                                     start=(k == 0), stop=False)
                for k in range(2):
                    nc.tensor.matmul(p[:, po : po + w], wap(5, k, g),
                                     rh_cur[k], start=False, stop=(k == 1))
                hh = wpool.tile([128, CH], f16, tag=f"hh{g}", name=f"hh{g}")
                nc.scalar.activation(hh[:, 0:w], p[:, po : po + w],
                                     AF.Tanh, bias=b_sb[:, 4 + g : 5 + g])
                o = combine_short(g, ut, m1, hh, w)
                orow = outT[g * 128 : (g + 1) * 128, :]
                eng = nc.scalar if g == 1 else nc.sync
                eng.dma_start(orow[:, c0 : c0 + w], o[:, 0:w])

    nc.compile()
    return nc


_NC_CACHE = {}


def _get_nc():
    key = (S, os.environ.get("GRU_MM_DTYPE", "float16"),
           os.environ.get("GRU_WARMUP", "10"))
    if key not in _NC_CACHE:
        _NC_CACHE[key] = build_nc(S, key[1])
    return _NC_CACHE[key]


def _make_in_maps(inputs):
    f32 = np.float32
    dt16 = {"float16": np.float16}.get(
        os.environ.get("GRU_MM_DTYPE", "float16")
    )
    if dt16 is None:
        import ml_dtypes

        dt16 = ml_dtypes.bfloat16
    x = np.asarray(inputs["x"], f32)
    h = np.asarray(inputs["h_t_1"], f32)
    wcat = np.ascontiguousarray(
        np.concatenate(
            [np.asarray(inputs[n], f32) for n in ("Wr", "Ur", "Wz", "Uz", "Wh", "Uh")],
            axis=1,
        ).astype(dt16)
    )
    bcat = np.ascontiguousarray(
        np.concatenate(
            [np.asarray(inputs[n], f32).reshape(2, 128).T for n in ("br", "bz", "bh")],
            axis=1,
        )
    )
    consts = {"wcat": wcat, "bcat": bcat}
    in_maps = []
    for c in range(N_CORES):
        sl = slice(c * S, (c + 1) * S)
        xT = x[sl].T.astype(dt16)  # [256, S]
        hT = h[sl].T.astype(dt16)
        xhm = np.empty((128, 4, S), dt16)
        xhm[:, 0] = xT[0:128]
        xhm[:, 1] = xT[128:256]
        xhm[:, 2] = hT[0:128]
        xhm[:, 3] = hT[128:256]
        m = {"xh": np.ascontiguousarray(xhm)}
        m.update(consts)
        in_maps.append(m)
    return in_maps


def run(inputs, trace=False):
    """Run on hardware; returns (h_t ndarray, BassKernelResults)."""
    from concourse.bass_utils import run_bass_kernel_spmd

    nc = _get_nc()
    in_maps = _make_in_maps(inputs)
    res = run_bass_kernel_spmd(nc, in_maps, list(range(N_CORES)), trace=trace)
    out = np.empty((B, D), np.float32)
    for c in range(N_CORES):
        out[c * S : (c + 1) * S] = res.results[c]["outT"].T.astype(np.float32)
    return out, res


def kernel(**inputs):
    out, _ = run(inputs, trace=False)
    return (out, out)


# revision 23
# speedup vs baseline: 1.0292x; 1.0292x over previous
"""GRU cell on 8 Trainium2 NeuronCores.

Reference computation (B=65536, D=256):
    z = sigmoid(x@Wz + h@Uz + bz)
    r = sigmoid(x@Wr + h@Ur + br)
    h_hat = tanh(x@Wh + (r*h)@Uh + bh)
    h_t = z*h + (1-z)*h_hat  ; returns (h_t, h_t)

Strategy: data-parallel over the batch dim (8 shards of 8192 rows), all
fp16 on chip (rel_l2 ~1.1e-3 vs the f32 reference; gate is 2e-2).  The
matmul stream runs at the fp16 PE issue floor (24 matmul passes over
8192 batch cols = 196608 PE cycles = 81.9us at 2.4GHz), so the
remaining time is the fixed framework preamble (~7.2us), the initial
HBM fill, and the drain tail + postamble.  Key structure:
  * host packs each shard as [128 partitions, 4 blocks, 8192] fp16
    (blocks = x k0, x k1, h k0, h k1): the contraction dim of all six
    GEMMs is the SBUF partition dim, fp16 halves HBM traffic and
    streams the PE at full rate with fast weight load.
  * all input tiles are SBUF-resident (8.4MB of 24MB); tiles are 512+
    cols so per-partition DMA lines stay at/above the 1KB efficiency
    knee (256-col tiles fragment to 512B lines and halve effective DMA
    bandwidth - measured).  Head DMAs are spread over the sync/scalar
    HWDGE rings (FIFO per ring, ~8 completion-sem lanes total) and
    gpsimd's SWDGE ring in need-order; the bulk stream self-paces on
    sync's lane rotation.
  * both ACT tables (sigmoid+tanh) are force-loaded via dummy
    activations right after scalar's head triggers - lazily, the tanh
    table load (1.28us) sits mid-queue and stalls the first r-sigmoid
    (measured as candidate-matmul stalls).
  * dummy warm-up matmuls during the head DMAs hold the PE's HAM clock
    gate at 2.4GHz so the real stream starts warm; ~3.4us of PE-busy
    is needed for the 1.2->2.4GHz flip, and any PE idle gap >~1us
    before the flip risks a cold stretch (measured: a 2us gap cost
    ~3us of half-rate matmuls).
  * the r-gate of work item i+1 is computed one iteration early so its
    sigmoid + r*h (ScalarE+VectorE) never gate the candidate matmuls.
  * the last 512 cols split into two 256-col pieces on disjoint PSUM
    regions; both use the short combine (u=z-1, m1=z*h precomputed on
    DVE, so only v=u*hh, o=m1-v remain after the tanh) and the last
    piece is emitted z-gate-first + candidate g-split so after the
    very last matmul only tanh(g1)+2 DVE ops+store remain.  Tail
    stores avoid scalar's queue except the final g1 (a 0.65us store
    trigger between the final tanhs measurably delays the drain).
"""

import os
import sys

for _p in ("/opt/trn_rl_repo", "/root/.axon_site/_ro/trn_rl_repo"):
    if os.path.isdir(_p) and _p not in sys.path:
        sys.path.append(_p)

import numpy as np

B = 65536
D = 256
N_CORES = 8
S = B // N_CORES  # batch rows per core
CH = 512  # batch columns per PSUM bank / compute sub-chunk

# Input-tile load plan: (col_start, width).  The first two are per-block
# (pipeline head fill); the rest are packed 4-block loads.
PLAN = [(0, 512), (512, 512), (1024, 512), (1536, 512)] + [
    (2048 + 1024 * i, 1024) for i in range(6)
]
_BLOCKS = ("x0", "x1", "h0", "h1")
_WORDER = ("Wr", "Ur", "Wz", "Uz", "Wh", "Uh")

# Work items: (dram col start, width, psum col offset).
WI = [(i * CH, CH, 0) for i in range(15)] + [
    (7680, 256, 0),
    (7936, 256, 256),
]


def build_nc(s=S, mm_dtype_name=None):
    """Build + compile the per-core Bass program for a shard of s rows."""
    import concourse.bass as bass
    import concourse.mybir as mybir
    import concourse.tile as tile
    from concourse import bacc

    f32 = mybir.dt.float32
    if mm_dtype_name is None:
        mm_dtype_name = os.environ.get("GRU_MM_DTYPE", "float16")
    f16 = getattr(mybir.dt, mm_dtype_name)
    AF = mybir.ActivationFunctionType
    n_warm = int(os.environ.get("GRU_WARMUP", "10"))

    nc = bacc.Bacc("TRN2", target_bir_lowering=False)
    xh = nc.dram_tensor("xh", [128, 4, s], f16, kind="ExternalInput")
    wcat = nc.dram_tensor("wcat", [D, 6 * D], f16, kind="ExternalInput")
    bcat = nc.dram_tensor("bcat", [128, 6], f32, kind="ExternalInput")
    outT = nc.dram_tensor("outT", [D, s], f16, kind="ExternalOutput")

    nwi = len(WI)

    with tile.TileContext(nc) as tc:
        with (
            tc.tile_pool(name="const", bufs=1) as cpool,
            tc.tile_pool(name="work", bufs=2) as wpool,
            tc.tile_pool(name="outb", bufs=4) as opool,
            tc.tile_pool(name="psum", bufs=1, space=bass.MemorySpace.PSUM) as ppool,
        ):
            inp = {}  # (block, load_idx) -> AP [128, width]

            # PE warm-up: the HAM clock gate needs ~3.4us of sustained PE
            # activity to lift the engine from 1.2 to 2.4 GHz.  The PE is
            # idle during the head DMAs anyway, so burn that window on
            # dummy matmuls over a memset tile.
            zt0 = cpool.tile([128, CH], f16, tag="warm", name="warm")
            nc.gpsimd.memset(zt0[:], 0)
            pw = ppool.tile([128, CH], f32, tag="pwarm", name="pwarm")
            for _ in range(n_warm):
                nc.tensor.matmul(pw[:], zt0[:, 0:128], zt0[:], start=True, stop=True)
            wsink = cpool.tile([128, CH], f32, tag="wsink", name="wsink")
            nc.vector.tensor_copy(wsink[:], pw[:])

            # Head DMA scheduling: HWDGE rings are FIFO per issuing engine
            # with ~8 completion-sem lanes; a trigger past that blocks its
            # engine queue until an earlier DMA completes.  The first slots
            # carry exactly the critical set (r-gate weights + j=0 input
            # tiles); h tiles and stragglers ride GpSimd's SWDGE ring; the
            # bulk stream self-paces on sync's lane rotation.
            wA, wZ, wH = {}, {}, {}
            for k in range(2):
                wA[k] = cpool.tile([128, 2 * D], f16, tag=f"wA{k}", name=f"wA{k}")
                wZ[k] = cpool.tile([128, 2 * D], f16, tag=f"wZ{k}", name=f"wZ{k}")
                wH[k] = cpool.tile([128, 2 * D], f16, tag=f"wH{k}", name=f"wH{k}")

            def load_block(blk, li, eng):
                bi = _BLOCKS.index(blk)
                start, width = PLAN[li]
                t = cpool.tile([128, width], f16, tag=f"i{blk}_{li}",
                               name=f"i{blk}_{li}")
                eng.dma_start(t[:], xh[:, bi, start : start + width])
                inp[(blk, li)] = t[:]

            # wave 1: r-gate weights and the j=0 input tiles - exactly what
            # the first matmuls consume - plus the tiny bias vector
            nc.sync.dma_start(wA[0][:], wcat[0:128, 0 : 2 * D])
            nc.scalar.dma_start(wA[1][:], wcat[128:256, 0 : 2 * D])
            load_block("x0", 0, nc.sync)
            load_block("x1", 0, nc.scalar)
            load_block("h0", 0, nc.gpsimd)
            load_block("h1", 0, nc.gpsimd)
            b_sb = cpool.tile([128, 6], f32, tag="bcat")
            nc.sync.dma_start(b_sb[:], bcat[:, :])
            # z/candidate weights split per-gate so the z chunk lands
            # before the z matmuls need it (a single 4D-wide wB chunk
            # finishes ~2us after the z matmuls want its first half).
            # Both wH chunks ride scalar: its HWDGE ring is consistently
            # ~1.5x faster than sync's, and wH as sync's 5th entry missed
            # its deadline by 1.5-3us across runs.
            nc.scalar.dma_start(wZ[0][:], wcat[0:128, 2 * D : 4 * D])
            nc.sync.dma_start(wZ[1][:], wcat[128:256, 2 * D : 4 * D])
            nc.scalar.dma_start(wH[0][:], wcat[0:128, 4 * D : 6 * D])
            nc.scalar.dma_start(wH[1][:], wcat[128:256, 4 * D : 6 * D])
            # force both ACT tables (sigmoid + tanh) to load now, after
            # scalar's immediate triggers but BEFORE the rotation-blocked
            # stragglers (which hold scalar's queue until earlier DMAs
            # complete): lazily the tanh table load lands after them and
            # stalls the first r-sigmoid by ~1.5us
            dume = cpool.tile([128, 1], f16, tag="dume", name="dume")
            nc.scalar.activation(dume[:], zt0[:, 0:1], AF.Sigmoid)
            nc.scalar.activation(dume[:], zt0[:, 0:1], AF.Tanh)
            # The j=1 x tiles feed r1 at ~15.5us (uniform prefetch ladder);
            # rotation-paced on sync/scalar they land 16-18us (measured
            # 2.6us stall).  GpSimd's SWDGE ring is empty after the h j=0
            # tiles, so they ride there FIFO and land ~14-15us.  The j=1 h
            # tiles (needed ~16.5us) stay rotation-paced: they throttle
            # themselves on their engine's completion-sem lanes and cannot
            # steal HBM bandwidth from the critical pieces above.
            load_block("x0", 1, nc.gpsimd)
            load_block("x1", 1, nc.gpsimd)
            load_block("h0", 1, nc.sync)
            load_block("h1", 1, nc.scalar)
            for li in range(2, len(PLAN)):
                start, width = PLAN[li]
                t = cpool.tile([128, 4, width], f16, tag=f"ixh_{li}",
                               name=f"ixh_{li}")
                nc.sync.dma_start(t[:], xh[:, :, start : start + width])
                for bi, blk in enumerate(_BLOCKS):
                    inp[(blk, li)] = t[:, bi, :]

            def wap(i, k, g):
                """Weight AP [128,128] for matrix index i (order _WORDER),
                contraction half k, output-feature half g."""
                chunk = (wA, wA, wZ, wZ, wH, wH)[i]
                return chunk[k][:, (i % 2) * D + g * 128 : (i % 2) * D + (g + 1) * 128]

            def inp_ap(blk, c0, w):
                for li, (start, width) in enumerate(PLAN):
                    if start <= c0 and c0 + w <= start + width:
                        return inp[(blk, li)][:, c0 - start : c0 - start + w]
                raise ValueError((blk, c0, w))

            def operands(i):
                c0, w, _ = WI[i]
                xs = [inp_ap(f"x{k}", c0, w) for k in range(2)]
                hs = [inp_ap(f"h{k}", c0, w) for k in range(2)]
                return xs, hs

            def gate_pair(tagbase, wi, ui, xs, rhs_u, po, w):
                """Both g-halves of one gate.  W (x-side) matmuls of both
                halves run before the U matmuls; k-major within each pass."""
                ps = []
                for g in range(2):
                    p = ppool.tile([128, CH], f32, tag=f"{tagbase}{g}",
                                   name=f"{tagbase}{g}")
                    ps.append(p)
                for k in range(2):
                    for g in range(2):
                        nc.tensor.matmul(ps[g][:, po : po + w], wap(wi, k, g),
                                         xs[k], start=(k == 0), stop=False)
                for k in range(2):
                    for g in range(2):
                        nc.tensor.matmul(ps[g][:, po : po + w], wap(ui, k, g),
                                         rhs_u[k], start=False, stop=(k == 1))
                return ps

            def r_gate(i):
                """reset gate -> r*h tiles for work item i."""
                c0, w, po = WI[i]
                xs, hs = operands(i)
                ps = gate_pair("pr", 0, 1, xs, hs, po, w)
                rh = []
                for g in range(2):
                    rt = wpool.tile([128, CH], f16, tag=f"r{g}", name=f"r{g}")
                    nc.scalar.activation(rt[:, 0:w], ps[g][:, po : po + w],
                                         AF.Sigmoid, bias=b_sb[:, g : g + 1])
                    t = wpool.tile([128, CH], f16, tag=f"rh{g}", name=f"rh{g}")
                    nc.vector.tensor_mul(t[:, 0:w], rt[:, 0:w], hs[g])
                    rh.append(t[:, 0:w])
                return rh

            def zu_gate(i, emit_um):
                """z-sigmoids for item i; with emit_um also u=z-1, m1=z*h
                on DVE (NOT GpSimd: its tensor ops cost ~3.8us each plus a
                9us drain penalty - measured) so only two DVE links remain
                after the final tanh: o = z*h+(1-z)*hh = m1-u*hh."""
                c0, w, po = WI[i]
                xs, hs = operands(i)
                pz = gate_pair("pz", 2, 3, xs, hs, po, w)
                zt, ut, m1 = [], [], []
                for g in range(2):
                    t = wpool.tile([128, CH], f16, tag=f"z{g}", name=f"z{g}")
                    nc.scalar.activation(t[:, 0:w], pz[g][:, po : po + w],
                                         AF.Sigmoid, bias=b_sb[:, 2 + g : 3 + g])
                    zt.append(t)
                    if emit_um:
                        u = wpool.tile([128, CH], f16, tag=f"u{g}", name=f"u{g}")
                        nc.vector.tensor_scalar_sub(u[:, 0:w], t[:, 0:w], 1.0)
                        ut.append(u)
                        m = wpool.tile([128, CH], f16, tag=f"zh{g}", name=f"zh{g}")
                        nc.vector.tensor_mul(m[:, 0:w], t[:, 0:w], hs[g])
                        m1.append(m)
                return zt, ut, m1

            def combine_short(g, ut, m1, hh, w):
                v = wpool.tile([128, CH], f16, tag=f"v{g}", name=f"v{g}")
                nc.vector.tensor_mul(v[:, 0:w], ut[g][:, 0:w], hh[:, 0:w])
                o = opool.tile([128, CH], f16, tag=f"o{g}", name=f"o{g}")
                nc.vector.tensor_sub(o[:, 0:w], m1[g][:, 0:w], v[:, 0:w])
                return o

            # software pipeline: r-gate one work item ahead of z/candidate;
            # per-iteration order is z_i, r_{i+1}, h_i so the r_i sigmoid +
            # r*h chain gets a 16-matmul window before h_i consumes it
            # (uniform from i=0 on - prefetching r_1 inside item 0 shrinks
            # the pipeline-fill bubble by ~1us).
            rh_cur = r_gate(0)
            for i in range(nwi - 1):
                c0, w, po = WI[i]
                xs, hs = operands(i)
                tail = i == nwi - 2
                zt, ut, m1 = zu_gate(i, False)
                rh_next = r_gate(i + 1) if i + 1 < nwi else None
                ph = gate_pair("ph", 4, 5, xs, rh_cur, po, w)
                for g in range(2):
                    hh = wpool.tile([128, CH], f16, tag=f"hh{g}", name=f"hh{g}")
                    nc.scalar.activation(hh[:, 0:w], ph[g][:, po : po + w],
                                         AF.Tanh, bias=b_sb[:, 4 + g : 5 + g])
                    o = opool.tile([128, CH], f16, tag=f"o{g}", name=f"o{g}")
                    d = wpool.tile([128, CH], f16, tag=f"d{g}", name=f"d{g}")
                    nc.vector.tensor_sub(d[:, 0:w], hs[g], hh[:, 0:w])
                    m = wpool.tile([128, CH], f16, tag=f"m{g}", name=f"m{g}")
                    nc.vector.tensor_mul(m[:, 0:w], zt[g][:, 0:w], d[:, 0:w])
                    nc.vector.tensor_add(o[:, 0:w], hh[:, 0:w], m[:, 0:w])
                    orow = outT[g * 128 : (g + 1) * 128, :]
                    # bulk stores ride gpsimd's SWDGE ring.  The second-to-
                    # last item's g0 goes on sync, g1 on gpsimd: both sync-
                    # both would rotation-block sync's queue ahead of the
                    # LAST item's store trigger (measured +1.5us), and
                    # scalar must stay clear for the final tanhs.
                    if tail:
                        eng = nc.sync if g == 0 else nc.gpsimd
                    else:
                        eng = nc.gpsimd
                    eng.dma_start(orow[:, c0 : c0 + w], o[:, 0:w])
                rh_cur = rh_next

            # Last item, drain-optimized: z first (sigmoid + u/m1 while the
            # candidate matmuls run), candidate g-split so after the very
            # last matmul only tanh(g1) -> v -> o -> store remain.
            i = nwi - 1
            c0, w, po = WI[i]
            xs, hs = operands(i)
            zt, ut, m1 = zu_gate(i, True)
            for g in range(2):
                p = ppool.tile([128, CH], f32, tag=f"ph{g}", name=f"ph{g}")
                for k in range(2):
                    nc.tensor.matmul(p[:, po : po + w], wap(4, k, g), xs[k],
                                     start=(k == 0), stop=False)
                for k in range(2):
                    nc.tensor.matmul(p[:, po : po + w], wap(5, k, g),
                                     rh_cur[k], start=False, stop=(k == 1))
                hh = wpool.tile([128, CH], f16, tag=f"hh{g}", name=f"hh{g}")
                nc.scalar.activation(hh[:, 0:w], p[:, po : po + w],
                                     AF.Tanh, bias=b_sb[:, 4 + g : 5 + g])
                o = combine_short(g, ut, m1, hh, w)
                orow = outT[g * 128 : (g + 1) * 128, :]
                eng = nc.scalar if g == 1 else nc.sync
                eng.dma_start(orow[:, c0 : c0 + w], o[:, 0:w])

    nc.compile()
    return nc


_NC_CACHE = {}


def _get_nc():
    key = (S, os.environ.get("GRU_MM_DTYPE", "float16"),
           os.environ.get("GRU_WARMUP", "10"))
    if key not in _NC_CACHE:
        _NC_CACHE[key] = build_nc(S, key[1])
    return _NC_CACHE[key]


def _make_in_maps(inputs):
    f32 = np.float32
    dt16 = {"float16": np.float16}.get(
        os.environ.get("GRU_MM_DTYPE", "float16")
    )
    if dt16 is None:
        import ml_dtypes

        dt16 = ml_dtypes.bfloat16
    x = np.asarray(inputs["x"], f32)
    h = np.asarray(inputs["h_t_1"], f32)
    wcat = np.ascontiguousarray(
        np.concatenate(
            [np.asarray(inputs[n], f32) for n in ("Wr", "Ur", "Wz", "Uz", "Wh", "Uh")],
            axis=1,
        ).astype(dt16)
    )
    bcat = np.ascontiguousarray(
        np.concatenate(
            [np.asarray(inputs[n], f32).reshape(2, 128).T for n in ("br", "bz", "bh")],
            axis=1,
        )
    )
    consts = {"wcat": wcat, "bcat": bcat}
    in_maps = []
    for c in range(N_CORES):
        sl = slice(c * S, (c + 1) * S)
        xT = x[sl].T.astype(dt16)  # [256, S]
        hT = h[sl].T.astype(dt16)
        xhm = np.empty((128, 4, S), dt16)
        xhm[:, 0] = xT[0:128]
        xhm[:, 1] = xT[128:256]
        xhm[:, 2] = hT[0:128]
        xhm[:, 3] = hT[128:256]
        m = {"xh": np.ascontiguousarray(xhm)}
        m.update(consts)
        in_maps.append(m)
    return in_maps


def run(inputs, trace=False):
    """Run on hardware; returns (h_t ndarray, BassKernelResults)."""
    from concourse.bass_utils import run_bass_kernel_spmd

    nc = _get_nc()
    in_maps = _make_in_maps(inputs)
    res = run_bass_kernel_spmd(nc, in_maps, list(range(N_CORES)), trace=trace)
    out = np.empty((B, D), np.float32)
    for c in range(N_CORES):
        out[c * S : (c + 1) * S] = res.results[c]["outT"].T.astype(np.float32)
    return out, res


def kernel(**inputs):
    out, _ = run(inputs, trace=False)
    return (out, out)
